# revision 24
# baseline (speedup 1.0000x reference)
"""CrossModalAttention Trainium2 kernel.

Math (per batch):
  x  = concat([v @ Wv.T, l @ Wl.T], seq)          # [S=1024, E=1024]
  A~ = exp((x @ x.T) / sqrt(E))                   # unnormalized attn (symmetric scores)
  out = (A~ / rowsum(A~)) @ (x @ Wo.T + bo)       # bias fold: rows of attn sum to 1

Key tricks:
  - data-parallel over batch: 16 batches -> 2 per core, no collectives
  - host pre-transposes tokens ([b,s,d]->[b,d,s], concat v|l) and weights,
    so every matmul contraction dim is already on partitions: NO on-chip transposes
  - (attn @ x) @ Wo.T reassociated to attn @ (x @ Wo.T) so the PV matmul
    consumes z=[k,f] (natural layout from an e-contraction) instead of x-natural
  - scores matrix is symmetric => exp(scores) tiles serve both as
    [k-part, q-free] (PV lhsT) and [q-part, k-free] (softmax row-sum on DVE)
  - softmax max-subtraction skipped: |scores/32| <= ~16 for this data; a
    constant bias inside exp (cancelled exactly by normalization) keeps the
    fp16 probabilities in range
  - fp16 matmuls by default (1 cycle/row, fp32 PSUM accumulate; ~5e-4 rel
    error vs fp32 reference). Alternatives kept for fallback: f32r (~tf32,
    2.6e-4, ~1.7x slower: each fp32r matmul pays an unoverlapped internal
    weight load) and exact fp32 (4 cycles/row).
"""

import numpy as np

B, SV, SL, E = 16, 576, 448, 1024
S = SV + SL  # 1024
NCORES = 8
BPC = B // NCORES  # batches per core
NT = 8  # 128-tiles per 1024 dim

_prog_cache = {}


ALL_STAGES = ("proj", "scores", "zs", "pv")


def _build_program(repeat=1, mm_mode="f32r", stages=ALL_STAGES):
    """Build the per-core Bass program. All cores run the same program (SPMD).

    mm_mode: "f8s" (fp16 + fp8-DoubleRow scores, fastest) / "f16" / "bf16" /
             "f32r" (~tf32) / "f32" (exact)
    repeat>1 wraps the body in a hardware loop (tc.For_i) for timing runs.
    stages: subset of ALL_STAGES for isolation benchmarks (outputs are garbage
    unless all stages are enabled).
    """
    import concourse.bacc as bacc
    import concourse.tile as tile
    import concourse.mybir as mybir

    dt = mybir.dt
    f32 = dt.float32
    use_fp8 = mm_mode in ("f8s", "f8p")
    fp8_proj = mm_mode == "f8p"  # proj in fp8-DR; z folded to tokens @ (Wo W)^T
    S8 = 8.0  # fp8 copy pre-scale (pushes values out of the subnormal range)
    SW8 = 64.0  # fp8 weight pre-scale for the proj weights (std 0.02 -> ~1.3)
    MDT = {"f32r": dt.float32r, "f32": dt.float32, "bf16": dt.bfloat16,
           "f16": dt.float16, "f8s": dt.float16, "f8p": dt.float16}[mm_mode]
    # fp16 can't hold exp(score) for scores up to ~+15; shift the exponent by a
    # constant. Softmax normalization cancels any constant shift exactly, so
    # this changes nothing mathematically. With this data scores/sqrt(E) peak
    # ~13-15, so p~ stays in [e^-20, e^+12] -> safe in fp16 up to score ~+25.
    EXP_BIAS = -14.0 if mm_mode in ("f16", "f8s", "f8p") else 0.0
    AF = mybir.ActivationFunctionType
    AX = mybir.AxisListType

    nc = bacc.Bacc("TRN2", target_bir_lowering=False, debug=False, enable_asserts=True)

    xt_ap = nc.dram_tensor("xt", [BPC, E, S], MDT, kind="ExternalInput").ap()
    if fp8_proj:
        # fp8 tokens in DoubleRow d-pair layout + fp8 proj weights; z weights
        # pre-folded with Wo on the host: wvot = (Wo @ Wv).T etc.
        xt8_ap = nc.dram_tensor("xt8", [BPC, NT // 2, 128, 2, S], dt.float8e4,
                                kind="ExternalInput").ap()
        wv8_ap = nc.dram_tensor("wv8", [NT // 2, 128, 2, E], dt.float8e4,
                                kind="ExternalInput").ap()
        wl8_ap = nc.dram_tensor("wl8", [NT // 2, 128, 2, E], dt.float8e4,
                                kind="ExternalInput").ap()
        wvot_ap = nc.dram_tensor("wvot", [E, E], MDT, kind="ExternalInput").ap()
        wlot_ap = nc.dram_tensor("wlot", [E, E], MDT, kind="ExternalInput").ap()
    else:
        wvt_ap = nc.dram_tensor("wvt", [E, E], MDT, kind="ExternalInput").ap()
        wlt_ap = nc.dram_tensor("wlt", [E, E], MDT, kind="ExternalInput").ap()
        wot_ap = nc.dram_tensor("wot", [E, E], MDT, kind="ExternalInput").ap()
    bob_ap = nc.dram_tensor("bob", [128, E], f32, kind="ExternalInput").ap()
    out_ap = nc.dram_tensor("out", [BPC, S, E], f32, kind="ExternalOutput").ap()

    # proj output column chunks: [start, width, which-weight]
    PROJ_CHUNKS = [(0, 288, "v"), (288, 288, "v"), (576, 448, "l")]
    # z (= tokens @ (Wo W)^T) s-row tiles: which folded weight each kt uses;
    # kt 4 straddles the vision/language boundary at s=576 and is split into
    # two half-partition groups (concurrent via PE column tiling).
    Z_TILES = [(kt, [("v" if kt < 4 else "l", 0, 128)]) if kt != 4 else
               (4, [("v", 0, 64), ("l", 64, 64)]) for kt in range(NT)]

    with tile.TileContext(nc) as tc:
        import contextlib

        with contextlib.ExitStack() as ctx:
            p_wo = ctx.enter_context(tc.tile_pool(name="wo", bufs=1))
            p_bo = ctx.enter_context(tc.tile_pool(name="bo", bufs=1))
            p_tok = ctx.enter_context(tc.tile_pool(name="tok", bufs=1))
            p_wst = ctx.enter_context(tc.tile_pool(name="wst", bufs=3))
            p_xT = ctx.enter_context(tc.tile_pool(name="xT", bufs=1))
            p_x8 = ctx.enter_context(tc.tile_pool(name="x8", bufs=1))
            p_exp = ctx.enter_context(tc.tile_pool(name="expT", bufs=1))
            p_z = ctx.enter_context(tc.tile_pool(name="z", bufs=1))
            p_sm = ctx.enter_context(tc.tile_pool(name="sm", bufs=1))
            p_out = ctx.enter_context(tc.tile_pool(name="outs", bufs=4))
            p_ps = ctx.enter_context(tc.tile_pool(name="ps", bufs=8, space="PSUM"))

            # --- one-time loads: weights resident, bias broadcast ---
            # 2-byte modes: everything fits, keep Wv.T/Wl.T resident too and
            # double-buffer token tiles (no per-batch weight re-DMA at all).
            two_byte = MDT in (dt.bfloat16, dt.float16)
            wres = {}
            if fp8_proj:
                # folded z weights (Wo W).T resident per modality
                wz = {}
                for wkey, wap in (("v", wvot_ap), ("l", wlot_ap)):
                    tiles = []
                    for d in range(NT):
                        w = p_wo.tile([128, E], MDT, tag=f"wz{wkey}{d}",
                                      name=f"wz{wkey}{d}")
                        nc.sync.dma_start(w[:], wap[d * 128:(d + 1) * 128, :])
                        tiles.append(w)
                    wz[wkey] = tiles
                # fp8 proj weights in DR pair layout
                w8 = {}
                for wkey, wap in (("v", wv8_ap), ("l", wl8_ap)):
                    tiles = []
                    for j in range(NT // 2):
                        w = p_wo.tile([128, 2, E], dt.float8e4,
                                      tag=f"w8{wkey}{j}", name=f"w8{wkey}{j}")
                        nc.sync.dma_start(w[:], wap[j])
                        tiles.append(w)
                    w8[wkey] = tiles
            else:
                wot_s = []
                for e in range(NT):
                    w = p_wo.tile([128, E], MDT, tag=f"wo{e}", name=f"wot{e}")
                    nc.sync.dma_start(w[:], wot_ap[e * 128:(e + 1) * 128, :])
                    wot_s.append(w)
                if two_byte:
                    for wkey, wap in (("v", wvt_ap), ("l", wlt_ap)):
                        tiles = []
                        for d in range(NT):
                            w = p_wo.tile([128, E], MDT, tag=f"w{wkey}r{d}",
                                          name=f"w{wkey}res{d}")
                            nc.sync.dma_start(w[:],
                                              wap[d * 128:(d + 1) * 128, :])
                            tiles.append(w)
                        wres[wkey] = tiles
            bo_b = p_bo.tile([128, E], f32, tag="bo", name="bo_b")
            nc.sync.dma_start(bo_b[:], bob_ap[:])
            expb = None
            if EXP_BIAS != 0.0:
                expb = p_bo.tile([128, 1], f32, tag="expb", name="expb")
                nc.gpsimd.memset(expb[:], EXP_BIAS)



            def body():
                for b in range(BPC):
                    # --- per-batch tile allocations (shared across stages) ---
                    xtok = [p_tok.tile([128, S], MDT, tag=f"tok{d}",
                                       bufs=(2 if two_byte else 1),
                                       name=f"tok{b}_{d}")
                            for d in range(NT)]
                    tok8 = []
                    if fp8_proj:
                        tok8 = [p_tok.tile([128, 2, S], dt.float8e4,
                                           tag=f"tok8_{j}", bufs=2,
                                           name=f"tok8_{b}_{j}")
                                for j in range(NT // 2)]
                    x8 = []
                    if use_fp8:
                        x8 = [p_x8.tile([128, 2, S], dt.float8e4,
                                        tag=f"x8_{j}", bufs=2,
                                        name=f"x8_{b}_{j}")
                              for j in range(NT // 2)]
                    xT = []
                    if not fp8_proj:
                        xT = [p_xT.tile([128, S], MDT, tag=f"xT{e}",
                                        bufs=(2 if two_byte else 1),
                                        name=f"xT{b}_{e}")
                              for e in range(NT)]
                    expT = [p_exp.tile([128, S], MDT, tag=f"ex{i}",
                                       bufs=(2 if two_byte else 1),
                                       name=f"ex{b}_{i}")
                            for i in range(NT)]
                    # zs bufs=1: PV(b) precedes zs(b+1) in PE program order, so
                    # single-buffering costs nothing and saves 16KB/partition.
                    zs = {fc: [p_z.tile([128, 512], MDT, tag=f"z{fc}_{kt}",
                                        bufs=1, name=f"z{b}_{fc}_{kt}")
                               for kt in range(NT)]
                          for fc in range(2)}
                    recs = [p_sm.tile([128, 1], f32, tag=f"rec{i}",
                                      name=f"rc{b}_{i}")
                            for i in range(NT)]

                    # stage isolation: zero-fill inputs whose producer stage
                    # is disabled (gpsimd; off the critical engines)
                    if "scores" in stages and "proj" not in stages and use_fp8:
                        for j in range(NT // 2):
                            nc.gpsimd.memset(x8[j][:], 0.0)
                    if "proj" not in stages and (
                            "zs" in stages or
                            ("scores" in stages and not use_fp8)):
                        # DMA (idle engines) rather than gpsimd memset; the
                        # values don't matter for timing, the bytes fit.
                        for e in range(NT):
                            if fp8_proj:
                                nc.sync.dma_start(
                                    xtok[e][:],
                                    xt_ap[b, e * 128:(e + 1) * 128, :])
                            else:
                                nc.sync.dma_start(
                                    xT[e][:],
                                    xt_ap[b, e * 128:(e + 1) * 128, :])
                    if "pv" in stages and "scores" not in stages:
                        for i in range(NT):
                            nc.gpsimd.memset(expT[i][:], 0.0)
                            nc.gpsimd.memset(recs[i][:], 1.0)
                    if "pv" in stages and "zs" not in stages:
                        for fc in range(2):
                            for kt in range(NT):
                                nc.gpsimd.memset(zs[fc][kt][:], 0.0)

                    # --- proj: load tokens, project -> x (for the scores path)
                    # f8s: fp16 proj -> xT fp16 + fp8 copy x8.
                    # f8p: fp8-DR proj (tokens and weights fp8) -> x8 only;
                    #      xT not needed (z is folded to tokens @ (Wo W).T).
                    if "proj" in stages:
                        for d in range(NT):
                            nc.sync.dma_start(
                                xtok[d][:], xt_ap[b, d * 128:(d + 1) * 128, :])
                        if fp8_proj:
                            for j in range(NT // 2):
                                nc.sync.dma_start(tok8[j][:], xt8_ap[b, j])
                            for e in range(NT):
                                for cs, cw, wkey in PROJ_CHUNKS:
                                    ps = p_ps.tile([128, 512], f32, tag="ps",
                                                   name=f"psp{b}_{e}_{cs}")
                                    for j in range(NT // 2):
                                        nc.tensor.matmul(
                                            ps[:, :cw],
                                            w8[wkey][j][:, :,
                                                        e * 128:(e + 1) * 128],
                                            tok8[j][:, :, cs:cs + cw],
                                            start=(j == 0),
                                            stop=(j == NT // 2 - 1),
                                            perf_mode=mybir.MatmulPerfMode.DoubleRow)
                                    nc.scalar.activation(
                                        x8[e // 2][:, e % 2, cs:cs + cw],
                                        ps[:, :cw], AF.Copy, scale=S8 / SW8)
                        else:
                            for e in range(NT):
                                if two_byte:
                                    stripes = {
                                        k: [wres[k][d][:, e * 128:(e + 1) * 128]
                                            for d in range(NT)]
                                        for k in ("v", "l")}
                                else:
                                    stripes = {"v": [], "l": []}
                                    for wkey, wap in (("v", wvt_ap),
                                                      ("l", wlt_ap)):
                                        for d in range(NT):
                                            w = p_wst.tile(
                                                [128, 128], MDT,
                                                tag=f"w{wkey}{d}",
                                                name=f"w{wkey}{b}_{e}_{d}")
                                            nc.sync.dma_start(
                                                w[:],
                                                wap[d * 128:(d + 1) * 128,
                                                    e * 128:(e + 1) * 128])
                                            stripes[wkey].append(w[:])
                                for cs, cw, wkey in PROJ_CHUNKS:
                                    ps = p_ps.tile([128, 512], f32, tag="ps",
                                                   name=f"psp{b}_{e}_{cs}")
                                    for d in range(NT):
                                        nc.tensor.matmul(
                                            ps[:, :cw], stripes[wkey][d],
                                            xtok[d][:, cs:cs + cw],
                                            start=(d == 0), stop=(d == NT - 1))
                                    nc.vector.tensor_copy(xT[e][:, cs:cs + cw],
                                                          ps[:, :cw])
                                    if use_fp8:
                                        nc.scalar.activation(
                                            x8[e // 2][:, e % 2, cs:cs + cw],
                                            ps[:, :cw], AF.Copy, scale=S8)

                    # --- scores + exp (scale folded into activation);
                    # rowsums come free from the exp's accum_out.
                    if "scores" in stages:
                        sc_scale = float(E) ** -0.5 / (S8 * S8 if use_fp8 else 1.0)
                        sumhalf = []
                        for i in range(NT):
                            sh = []
                            for jc in range(2):
                                ps = p_ps.tile([128, 512], f32, tag="ps",
                                               name=f"pss{b}_{i}_{jc}")
                                if use_fp8:
                                    for j in range(NT // 2):
                                        nc.tensor.matmul(
                                            ps[:],
                                            x8[j][:, :, i * 128:(i + 1) * 128],
                                            x8[j][:, :, jc * 512:(jc + 1) * 512],
                                            start=(j == 0),
                                            stop=(j == NT // 2 - 1),
                                            perf_mode=mybir.MatmulPerfMode.DoubleRow)
                                else:
                                    for e in range(NT):
                                        nc.tensor.matmul(
                                            ps[:], xT[e][:, i * 128:(i + 1) * 128],
                                            xT[e][:, jc * 512:(jc + 1) * 512],
                                            start=(e == 0), stop=(e == NT - 1))
                                acc = p_sm.tile([128, 1], f32, tag=f"acc{i}_{jc}",
                                                name=f"acc{b}_{i}_{jc}")
                                nc.scalar.activation(
                                    expT[i][:, jc * 512:(jc + 1) * 512], ps[:],
                                    AF.Exp, scale=sc_scale,
                                    bias=(expb[:] if expb is not None else 0.0),
                                    accum_out=acc[:])
                                sh.append(acc)
                            sumhalf.append(sh)
                        for i in range(NT):
                            sums = p_sm.tile([128, 1], f32, tag=f"sum{i}",
                                             name=f"sm{b}_{i}")
                            nc.vector.tensor_add(sums[:], sumhalf[i][0][:],
                                                 sumhalf[i][1][:])
                            nc.vector.reciprocal(recs[i][:], sums[:])

                    # --- z + bo (both f-chunks) ---
                    # f8p: z = tokens @ (Wo W).T straight from fp16 tokens;
                    # the kt=4 s-tile straddles the modality boundary and is
                    # split into two concurrent half-partition groups.
                    # else: z = xT @ Wo.T.
                    if "zs" in stages:
                        for fc in range(2):
                            for kt, parts in (Z_TILES if fp8_proj else
                                              [(kt, None) for kt in range(NT)]):
                                ps = p_ps.tile([128, 512], f32, tag="ps",
                                               name=f"psz{b}_{fc}_{kt}")
                                if fp8_proj:
                                    for wkey, po, pw in parts:
                                        for d in range(NT):
                                            nc.tensor.matmul(
                                                ps[po:po + pw, :],
                                                xtok[d][:, kt * 128 + po:
                                                        kt * 128 + po + pw],
                                                wz[wkey][d][:, fc * 512:
                                                            (fc + 1) * 512],
                                                start=(d == 0),
                                                stop=(d == NT - 1))
                                else:
                                    for e in range(NT):
                                        nc.tensor.matmul(
                                            ps[:],
                                            xT[e][:, kt * 128:(kt + 1) * 128],
                                            wot_s[e][:, fc * 512:(fc + 1) * 512],
                                            start=(e == 0), stop=(e == NT - 1))
                                nc.vector.tensor_add(
                                    zs[fc][kt][:], ps[:],
                                    bo_b[:, fc * 512:(fc + 1) * 512])

                    # --- out = attn @ z (+ row normalization), DMA out ---
                    if "pv" in stages:
                        for fc in range(2):
                            for q in range(NT):
                                ps = p_ps.tile([128, 512], f32, tag="ps",
                                               name=f"psf{b}_{fc}_{q}")
                                for kt in range(NT):
                                    nc.tensor.matmul(
                                        ps[:], expT[kt][:, q * 128:(q + 1) * 128],
                                        zs[fc][kt][:],
                                        start=(kt == 0), stop=(kt == NT - 1))
                                ot = p_out.tile([128, 512], f32, tag="out",
                                                name=f"o{b}_{fc}_{q}")
                                nc.scalar.activation(ot[:], ps[:], AF.Copy,
                                                     scale=recs[q][:])
                                nc.sync.dma_start(
                                    out_ap[b, q * 128:(q + 1) * 128,
                                           fc * 512:(fc + 1) * 512], ot[:])

            if repeat == 1:
                body()
            else:
                with tc.For_i(0, repeat, 1):
                    body()

    nc.compile()
    return nc


def _get_program(repeat=1, mm_mode="f32r", stages=ALL_STAGES):
    key = (repeat, mm_mode, tuple(stages))
    if key not in _prog_cache:
        _prog_cache[key] = _build_program(repeat, mm_mode, stages)
    return _prog_cache[key]


S8 = 8.0
SW8 = 64.0


def _pair8(a):
    """[D, N] -> [D/256, 128, 2, N] fp8 DoubleRow d-pair layout."""
    import ml_dtypes
    D, N = a.shape
    return np.ascontiguousarray(
        a.reshape(D // 256, 2, 128, N).transpose(0, 2, 1, 3)
        .astype(ml_dtypes.float8_e4m3fn))


def _host_prep(vision_tokens, language_tokens, Wv, Wl, Wo, bo, mm_mode="f32r"):
    if mm_mode == "bf16":
        import ml_dtypes
        mdt = ml_dtypes.bfloat16
    elif mm_mode in ("f16", "f8s", "f8p"):
        mdt = np.float16
    else:
        mdt = np.float32
    v = np.asarray(vision_tokens, dtype=np.float32)
    l = np.asarray(language_tokens, dtype=np.float32)
    xt = np.concatenate(
        [v.transpose(0, 2, 1), l.transpose(0, 2, 1)], axis=2
    )  # [B, E(d), S]
    xtc = np.ascontiguousarray(xt.astype(mdt))
    bob = np.ascontiguousarray(
        np.broadcast_to(np.asarray(bo, dtype=np.float32)[None, :], (128, E)))
    Wv32 = np.asarray(Wv, dtype=np.float32)
    Wl32 = np.asarray(Wl, dtype=np.float32)
    Wo32 = np.asarray(Wo, dtype=np.float32)
    if mm_mode == "f8p":
        xt8 = np.stack([_pair8(xt[b]) for b in range(B)])  # [B,4,128,2,S]
        wv8 = _pair8(Wv32.T * SW8)
        wl8 = _pair8(Wl32.T * SW8)
        wvot = np.ascontiguousarray((Wo32 @ Wv32).T.astype(mdt))
        wlot = np.ascontiguousarray((Wo32 @ Wl32).T.astype(mdt))
        return {"xt": xtc, "xt8": xt8, "wv8": wv8, "wl8": wl8,
                "wvot": wvot, "wlot": wlot, "bob": bob}
    wvt = np.ascontiguousarray(Wv32.T.astype(mdt))
    wlt = np.ascontiguousarray(Wl32.T.astype(mdt))
    wot = np.ascontiguousarray(Wo32.T.astype(mdt))
    return {"xt": xtc, "wvt": wvt, "wlt": wlt, "wot": wot, "bob": bob}


def _in_maps(tensors):
    per_batch = {"xt", "xt8"}
    return [
        {k: (v[c * BPC:(c + 1) * BPC] if k in per_batch else v)
         for k, v in tensors.items()}
        for c in range(NCORES)
    ]


DEFAULT_MODE = "f16"


def kernel(vision_tokens, language_tokens, Wv, Wl, Wo, bo):
    from concourse.bass_utils import run_bass_kernel_spmd

    tensors = _host_prep(
        vision_tokens, language_tokens, Wv, Wl, Wo, bo, DEFAULT_MODE)
    nc = _get_program(repeat=1, mm_mode=DEFAULT_MODE)
    res = run_bass_kernel_spmd(nc, _in_maps(tensors), list(range(NCORES)))
    out = np.concatenate([res.results[c]["out"] for c in range(NCORES)], axis=0)
    return np.ascontiguousarray(out.astype(np.float32))



# revision 27
# speedup vs baseline: 1.1707x; 1.1707x over previous
"""CrossModalAttention Trainium2 kernel.

Math (per batch):
  x  = concat([v @ Wv.T, l @ Wl.T], seq)          # [S=1024, E=1024]
  A~ = exp((x @ x.T) / sqrt(E))                   # unnormalized attn (symmetric scores)
  out = (A~ / rowsum(A~)) @ (x @ Wo.T + bo)       # bias fold: rows of attn sum to 1

Key tricks:
  - data-parallel over batch: 16 batches -> 2 per core, no collectives
  - host pre-transposes tokens ([b,s,d]->[b,d,s], concat v|l) and weights,
    so every matmul contraction dim is already on partitions: NO on-chip transposes
  - (attn @ x) @ Wo.T reassociated to attn @ (x @ Wo.T) so the PV matmul
    consumes z=[k,f] (natural layout from an e-contraction) instead of x-natural
  - scores matrix is symmetric => exp(scores) tiles serve both as
    [k-part, q-free] (PV lhsT) and [q-part, k-free] layouts
  - softmax row-sums come free from the exp activation's accum_out
  - softmax max-subtraction skipped: |scores/32| <= ~16 for this data; a
    constant bias inside exp (cancelled exactly by normalization) keeps the
    fp16 probabilities in range

Default mode "f8p" (HW-measured ~274us/iter vs ~388us for all-fp16 "f16"):
  - scores GEMM in fp8e4m3 DoubleRow (2 MACs/cell/cycle): softmax
    normalization absorbs the quantization (rel err ~8e-4 vs 5e-4 for fp16)
  - the value path must stay fp16 (fp8 there -> ~3e-2 absmax rel err), so
    Wo is folded into the projections ON HOST (Wvo = Wo@Wv, Wlo = Wo@Wl) and
    z = tokens @ Wvo.T comes straight from fp16 tokens - which frees the
    x-projection to run entirely in fp8 DoubleRow too (it only feeds scores)
  - the s=576 modality boundary splits one z row-tile into two
    half-partition groups, interleaved so PE column tiling runs them
    concurrently
  - stage order proj -> scores -> zs -> pv lets the exp/ACT tail hide
    behind the z GEMMs; all stages are PE-serial, total = sum of stage times
  - measurement note: the axon tunnel adds multi-second wall noise; only
    interleaved A/B with min-of-many differential timing is trustworthy

Fallback modes: "f8s" (fp8 scores only, ~345us), "f16" (~388us),
"bf16", "f32r" (~tf32), "f32" (exact).
"""

import numpy as np

B, SV, SL, E = 16, 576, 448, 1024
S = SV + SL  # 1024
NCORES = 8
BPC = B // NCORES  # batches per core
NT = 8  # 128-tiles per 1024 dim

_prog_cache = {}


ALL_STAGES = ("proj", "scores", "zs", "pv")


def _build_program(repeat=1, mm_mode="f32r", stages=ALL_STAGES):
    """Build the per-core Bass program. All cores run the same program (SPMD).

    mm_mode: "f8s" (fp16 + fp8-DoubleRow scores, fastest) / "f16" / "bf16" /
             "f32r" (~tf32) / "f32" (exact)
    repeat>1 wraps the body in a hardware loop (tc.For_i) for timing runs.
    stages: subset of ALL_STAGES for isolation benchmarks (outputs are garbage
    unless all stages are enabled).
    """
    import concourse.bacc as bacc
    import concourse.tile as tile
    import concourse.mybir as mybir

    dt = mybir.dt
    f32 = dt.float32
    use_fp8 = mm_mode in ("f8s", "f8p")
    fp8_proj = mm_mode == "f8p"  # proj in fp8-DR; z folded to tokens @ (Wo W)^T
    S8 = 8.0  # fp8 copy pre-scale (pushes values out of the subnormal range)
    SW8 = 64.0  # fp8 weight pre-scale for the proj weights (std 0.02 -> ~1.3)
    MDT = {"f32r": dt.float32r, "f32": dt.float32, "bf16": dt.bfloat16,
           "f16": dt.float16, "f8s": dt.float16, "f8p": dt.float16}[mm_mode]
    # fp16 can't hold exp(score) for scores up to ~+15; shift the exponent by a
    # constant. Softmax normalization cancels any constant shift exactly, so
    # this changes nothing mathematically. With this data scores/sqrt(E) peak
    # ~13-15, so p~ stays in [e^-20, e^+12] -> safe in fp16 up to score ~+25.
    EXP_BIAS = -14.0 if mm_mode in ("f16", "f8s", "f8p") else 0.0
    AF = mybir.ActivationFunctionType
    AX = mybir.AxisListType

    nc = bacc.Bacc("TRN2", target_bir_lowering=False, debug=False, enable_asserts=True)

    xt_ap = nc.dram_tensor("xt", [BPC, E, S], MDT, kind="ExternalInput").ap()
    if fp8_proj:
        # fp8 tokens in DoubleRow d-pair layout + fp8 proj weights; z weights
        # pre-folded with Wo on the host: wvot = (Wo @ Wv).T etc.
        xt8_ap = nc.dram_tensor("xt8", [BPC, NT // 2, 128, 2, S], dt.float8e4,
                                kind="ExternalInput").ap()
        wv8_ap = nc.dram_tensor("wv8", [NT // 2, 128, 2, E], dt.float8e4,
                                kind="ExternalInput").ap()
        wl8_ap = nc.dram_tensor("wl8", [NT // 2, 128, 2, E], dt.float8e4,
                                kind="ExternalInput").ap()
        wvot_ap = nc.dram_tensor("wvot", [E, E], MDT, kind="ExternalInput").ap()
        wlot_ap = nc.dram_tensor("wlot", [E, E], MDT, kind="ExternalInput").ap()
    else:
        wvt_ap = nc.dram_tensor("wvt", [E, E], MDT, kind="ExternalInput").ap()
        wlt_ap = nc.dram_tensor("wlt", [E, E], MDT, kind="ExternalInput").ap()
        wot_ap = nc.dram_tensor("wot", [E, E], MDT, kind="ExternalInput").ap()
    bob_ap = nc.dram_tensor("bob", [128, E], f32, kind="ExternalInput").ap()
    out_ap = nc.dram_tensor("out", [BPC, S, E], f32, kind="ExternalOutput").ap()

    # proj output column chunks: [start, width, which-weight]
    PROJ_CHUNKS = [(0, 288, "v"), (288, 288, "v"), (576, 448, "l")]
    # z (= tokens @ (Wo W)^T) s-row tiles: which folded weight each kt uses;
    # kt 4 straddles the vision/language boundary at s=576 and is split into
    # two half-partition groups (concurrent via PE column tiling).
    Z_TILES = [(kt, [("v" if kt < 4 else "l", 0, 128)]) if kt != 4 else
               (4, [("v", 0, 64), ("l", 64, 64)]) for kt in range(NT)]

    with tile.TileContext(nc) as tc:
        import contextlib

        with contextlib.ExitStack() as ctx:
            p_wo = ctx.enter_context(tc.tile_pool(name="wo", bufs=1))
            p_bo = ctx.enter_context(tc.tile_pool(name="bo", bufs=1))
            p_tok = ctx.enter_context(tc.tile_pool(name="tok", bufs=1))
            p_wst = ctx.enter_context(tc.tile_pool(name="wst", bufs=3))
            p_xT = ctx.enter_context(tc.tile_pool(name="xT", bufs=1))
            p_x8 = ctx.enter_context(tc.tile_pool(name="x8", bufs=1))
            p_exp = ctx.enter_context(tc.tile_pool(name="expT", bufs=1))
            p_z = ctx.enter_context(tc.tile_pool(name="z", bufs=1))
            p_sm = ctx.enter_context(tc.tile_pool(name="sm", bufs=1))
            p_out = ctx.enter_context(tc.tile_pool(name="outs", bufs=4))
            p_ps = ctx.enter_context(tc.tile_pool(name="ps", bufs=8, space="PSUM"))

            # --- one-time loads: weights resident, bias broadcast ---
            # 2-byte modes: everything fits, keep Wv.T/Wl.T resident too and
            # double-buffer token tiles (no per-batch weight re-DMA at all).
            two_byte = MDT in (dt.bfloat16, dt.float16)
            wres = {}
            if fp8_proj:
                # folded z weights (Wo W).T resident per modality
                wz = {}
                for wkey, wap in (("v", wvot_ap), ("l", wlot_ap)):
                    tiles = []
                    for d in range(NT):
                        w = p_wo.tile([128, E], MDT, tag=f"wz{wkey}{d}",
                                      name=f"wz{wkey}{d}")
                        nc.sync.dma_start(w[:], wap[d * 128:(d + 1) * 128, :])
                        tiles.append(w)
                    wz[wkey] = tiles
                # fp8 proj weights in DR pair layout
                w8 = {}
                for wkey, wap in (("v", wv8_ap), ("l", wl8_ap)):
                    tiles = []
                    for j in range(NT // 2):
                        w = p_wo.tile([128, 2, E], dt.float8e4,
                                      tag=f"w8{wkey}{j}", name=f"w8{wkey}{j}")
                        nc.sync.dma_start(w[:], wap[j])
                        tiles.append(w)
                    w8[wkey] = tiles
            else:
                wot_s = []
                for e in range(NT):
                    w = p_wo.tile([128, E], MDT, tag=f"wo{e}", name=f"wot{e}")
                    nc.sync.dma_start(w[:], wot_ap[e * 128:(e + 1) * 128, :])
                    wot_s.append(w)
                if two_byte:
                    for wkey, wap in (("v", wvt_ap), ("l", wlt_ap)):
                        tiles = []
                        for d in range(NT):
                            w = p_wo.tile([128, E], MDT, tag=f"w{wkey}r{d}",
                                          name=f"w{wkey}res{d}")
                            nc.sync.dma_start(w[:],
                                              wap[d * 128:(d + 1) * 128, :])
                            tiles.append(w)
                        wres[wkey] = tiles
            bo_b = p_bo.tile([128, E], f32, tag="bo", name="bo_b")
            nc.sync.dma_start(bo_b[:], bob_ap[:])
            expb = None
            if EXP_BIAS != 0.0:
                expb = p_bo.tile([128, 1], f32, tag="expb", name="expb")
                nc.gpsimd.memset(expb[:], EXP_BIAS)



            def body():
                for b in range(BPC):
                    # --- per-batch tile allocations (shared across stages) ---
                    xtok = [p_tok.tile([128, S], MDT, tag=f"tok{d}",
                                       bufs=(2 if two_byte else 1),
                                       name=f"tok{b}_{d}")
                            for d in range(NT)]
                    tok8 = []
                    if fp8_proj:
                        tok8 = [p_tok.tile([128, 2, S], dt.float8e4,
                                           tag=f"tok8_{j}", bufs=2,
                                           name=f"tok8_{b}_{j}")
                                for j in range(NT // 2)]
                    x8 = []
                    if use_fp8:
                        x8 = [p_x8.tile([128, 2, S], dt.float8e4,
                                        tag=f"x8_{j}", bufs=2,
                                        name=f"x8_{b}_{j}")
                              for j in range(NT // 2)]
                    xT = []
                    if not fp8_proj:
                        xT = [p_xT.tile([128, S], MDT, tag=f"xT{e}",
                                        bufs=(2 if two_byte else 1),
                                        name=f"xT{b}_{e}")
                              for e in range(NT)]
                    expT = [p_exp.tile([128, S], MDT, tag=f"ex{i}",
                                       bufs=(2 if two_byte else 1),
                                       name=f"ex{b}_{i}")
                            for i in range(NT)]
                    # zs bufs=1: PV(b) precedes zs(b+1) in PE program order, so
                    # single-buffering costs nothing and saves 16KB/partition.
                    zs = {fc: [p_z.tile([128, 512], MDT, tag=f"z{fc}_{kt}",
                                        bufs=1, name=f"z{b}_{fc}_{kt}")
                               for kt in range(NT)]
                          for fc in range(2)}
                    recs = [p_sm.tile([128, 1], f32, tag=f"rec{i}",
                                      name=f"rc{b}_{i}")
                            for i in range(NT)]

                    # stage isolation: zero-fill inputs whose producer stage
                    # is disabled (gpsimd; off the critical engines)
                    if "scores" in stages and "proj" not in stages and use_fp8:
                        for j in range(NT // 2):
                            nc.gpsimd.memset(x8[j][:], 0.0)
                    if "proj" not in stages and (
                            "zs" in stages or
                            ("scores" in stages and not use_fp8)):
                        # DMA (idle engines) rather than gpsimd memset; the
                        # values don't matter for timing, the bytes fit.
                        for e in range(NT):
                            if fp8_proj:
                                nc.sync.dma_start(
                                    xtok[e][:],
                                    xt_ap[b, e * 128:(e + 1) * 128, :])
                            else:
                                nc.sync.dma_start(
                                    xT[e][:],
                                    xt_ap[b, e * 128:(e + 1) * 128, :])
                    if "pv" in stages and "scores" not in stages:
                        for i in range(NT):
                            nc.gpsimd.memset(expT[i][:], 0.0)
                            nc.gpsimd.memset(recs[i][:], 1.0)
                    if "pv" in stages and "zs" not in stages:
                        for fc in range(2):
                            for kt in range(NT):
                                nc.gpsimd.memset(zs[fc][kt][:], 0.0)

                    # --- proj: load tokens, project -> x (for the scores path)
                    # f8s: fp16 proj -> xT fp16 + fp8 copy x8.
                    # f8p: fp8-DR proj (tokens and weights fp8) -> x8 only;
                    #      xT not needed (z is folded to tokens @ (Wo W).T).
                    if "proj" in stages:
                        for d in range(NT):
                            nc.sync.dma_start(
                                xtok[d][:], xt_ap[b, d * 128:(d + 1) * 128, :])
                        if fp8_proj:
                            for j in range(NT // 2):
                                nc.sync.dma_start(tok8[j][:], xt8_ap[b, j])
                            for e in range(NT):
                                for cs, cw, wkey in PROJ_CHUNKS:
                                    ps = p_ps.tile([128, 512], f32, tag="ps",
                                                   name=f"psp{b}_{e}_{cs}")
                                    for j in range(NT // 2):
                                        nc.tensor.matmul(
                                            ps[:, :cw],
                                            w8[wkey][j][:, :,
                                                        e * 128:(e + 1) * 128],
                                            tok8[j][:, :, cs:cs + cw],
                                            start=(j == 0),
                                            stop=(j == NT // 2 - 1),
                                            perf_mode=mybir.MatmulPerfMode.DoubleRow)
                                    nc.scalar.activation(
                                        x8[e // 2][:, e % 2, cs:cs + cw],
                                        ps[:, :cw], AF.Copy, scale=S8 / SW8)
                        else:
                            for e in range(NT):
                                if two_byte:
                                    stripes = {
                                        k: [wres[k][d][:, e * 128:(e + 1) * 128]
                                            for d in range(NT)]
                                        for k in ("v", "l")}
                                else:
                                    stripes = {"v": [], "l": []}
                                    for wkey, wap in (("v", wvt_ap),
                                                      ("l", wlt_ap)):
                                        for d in range(NT):
                                            w = p_wst.tile(
                                                [128, 128], MDT,
                                                tag=f"w{wkey}{d}",
                                                name=f"w{wkey}{b}_{e}_{d}")
                                            nc.sync.dma_start(
                                                w[:],
                                                wap[d * 128:(d + 1) * 128,
                                                    e * 128:(e + 1) * 128])
                                            stripes[wkey].append(w[:])
                                for cs, cw, wkey in PROJ_CHUNKS:
                                    ps = p_ps.tile([128, 512], f32, tag="ps",
                                                   name=f"psp{b}_{e}_{cs}")
                                    for d in range(NT):
                                        nc.tensor.matmul(
                                            ps[:, :cw], stripes[wkey][d],
                                            xtok[d][:, cs:cs + cw],
                                            start=(d == 0), stop=(d == NT - 1))
                                    nc.vector.tensor_copy(xT[e][:, cs:cs + cw],
                                                          ps[:, :cw])
                                    if use_fp8:
                                        nc.scalar.activation(
                                            x8[e // 2][:, e % 2, cs:cs + cw],
                                            ps[:, :cw], AF.Copy, scale=S8)

                    # --- scores + exp (scale folded into activation);
                    # rowsums come free from the exp's accum_out.
                    if "scores" in stages:
                        sc_scale = float(E) ** -0.5 / (S8 * S8 if use_fp8 else 1.0)
                        sumhalf = []
                        for i in range(NT):
                            sh = []
                            for jc in range(2):
                                ps = p_ps.tile([128, 512], f32, tag="ps",
                                               name=f"pss{b}_{i}_{jc}")
                                if use_fp8:
                                    for j in range(NT // 2):
                                        nc.tensor.matmul(
                                            ps[:],
                                            x8[j][:, :, i * 128:(i + 1) * 128],
                                            x8[j][:, :, jc * 512:(jc + 1) * 512],
                                            start=(j == 0),
                                            stop=(j == NT // 2 - 1),
                                            perf_mode=mybir.MatmulPerfMode.DoubleRow)
                                else:
                                    for e in range(NT):
                                        nc.tensor.matmul(
                                            ps[:], xT[e][:, i * 128:(i + 1) * 128],
                                            xT[e][:, jc * 512:(jc + 1) * 512],
                                            start=(e == 0), stop=(e == NT - 1))
                                acc = p_sm.tile([128, 1], f32, tag=f"acc{i}_{jc}",
                                                name=f"acc{b}_{i}_{jc}")
                                nc.scalar.activation(
                                    expT[i][:, jc * 512:(jc + 1) * 512], ps[:],
                                    AF.Exp, scale=sc_scale,
                                    bias=(expb[:] if expb is not None else 0.0),
                                    accum_out=acc[:])
                                sh.append(acc)
                            sumhalf.append(sh)
                        for i in range(NT):
                            sums = p_sm.tile([128, 1], f32, tag=f"sum{i}",
                                             name=f"sm{b}_{i}")
                            nc.vector.tensor_add(sums[:], sumhalf[i][0][:],
                                                 sumhalf[i][1][:])
                            nc.vector.reciprocal(recs[i][:], sums[:])

                    # --- z + bo (both f-chunks) ---
                    # f8p: z = tokens @ (Wo W).T straight from fp16 tokens;
                    # the kt=4 s-tile straddles the modality boundary and is
                    # split into two concurrent half-partition groups.
                    # else: z = xT @ Wo.T.
                    if "zs" in stages:
                        for fc in range(2):
                            for kt, parts in (Z_TILES if fp8_proj else
                                              [(kt, None) for kt in range(NT)]):
                                ps = p_ps.tile([128, 512], f32, tag="ps",
                                               name=f"psz{b}_{fc}_{kt}")
                                if fp8_proj:
                                    # interleave the (at most two) partition
                                    # groups so column-tiled matmuls overlap
                                    for d in range(NT):
                                        for wkey, po, pw in parts:
                                            nc.tensor.matmul(
                                                ps[po:po + pw, :],
                                                xtok[d][:, kt * 128 + po:
                                                        kt * 128 + po + pw],
                                                wz[wkey][d][:, fc * 512:
                                                            (fc + 1) * 512],
                                                start=(d == 0),
                                                stop=(d == NT - 1))
                                else:
                                    for e in range(NT):
                                        nc.tensor.matmul(
                                            ps[:],
                                            xT[e][:, kt * 128:(kt + 1) * 128],
                                            wot_s[e][:, fc * 512:(fc + 1) * 512],
                                            start=(e == 0), stop=(e == NT - 1))
                                nc.vector.tensor_add(
                                    zs[fc][kt][:], ps[:],
                                    bo_b[:, fc * 512:(fc + 1) * 512])

                    # --- out = attn @ z (+ row normalization), DMA out ---
                    if "pv" in stages:
                        for fc in range(2):
                            for q in range(NT):
                                ps = p_ps.tile([128, 512], f32, tag="ps",
                                               name=f"psf{b}_{fc}_{q}")
                                for kt in range(NT):
                                    nc.tensor.matmul(
                                        ps[:], expT[kt][:, q * 128:(q + 1) * 128],
                                        zs[fc][kt][:],
                                        start=(kt == 0), stop=(kt == NT - 1))
                                ot = p_out.tile([128, 512], f32, tag="out",
                                                name=f"o{b}_{fc}_{q}")
                                nc.scalar.activation(ot[:], ps[:], AF.Copy,
                                                     scale=recs[q][:])
                                nc.sync.dma_start(
                                    out_ap[b, q * 128:(q + 1) * 128,
                                           fc * 512:(fc + 1) * 512], ot[:])

            if repeat == 1:
                body()
            else:
                with tc.For_i(0, repeat, 1):
                    body()

    nc.compile()
    return nc


def _get_program(repeat=1, mm_mode="f32r", stages=ALL_STAGES):
    key = (repeat, mm_mode, tuple(stages))
    if key not in _prog_cache:
        _prog_cache[key] = _build_program(repeat, mm_mode, stages)
    return _prog_cache[key]


S8 = 8.0
SW8 = 64.0


def _pair8(a):
    """[D, N] -> [D/256, 128, 2, N] fp8 DoubleRow d-pair layout."""
    import ml_dtypes
    D, N = a.shape
    return np.ascontiguousarray(
        a.reshape(D // 256, 2, 128, N).transpose(0, 2, 1, 3)
        .astype(ml_dtypes.float8_e4m3fn))


def _host_prep(vision_tokens, language_tokens, Wv, Wl, Wo, bo, mm_mode="f32r"):
    if mm_mode == "bf16":
        import ml_dtypes
        mdt = ml_dtypes.bfloat16
    elif mm_mode in ("f16", "f8s", "f8p"):
        mdt = np.float16
    else:
        mdt = np.float32
    v = np.asarray(vision_tokens, dtype=np.float32)
    l = np.asarray(language_tokens, dtype=np.float32)
    xt = np.concatenate(
        [v.transpose(0, 2, 1), l.transpose(0, 2, 1)], axis=2
    )  # [B, E(d), S]
    xtc = np.ascontiguousarray(xt.astype(mdt))
    bob = np.ascontiguousarray(
        np.broadcast_to(np.asarray(bo, dtype=np.float32)[None, :], (128, E)))
    Wv32 = np.asarray(Wv, dtype=np.float32)
    Wl32 = np.asarray(Wl, dtype=np.float32)
    Wo32 = np.asarray(Wo, dtype=np.float32)
    if mm_mode == "f8p":
        xt8 = np.stack([_pair8(xt[b]) for b in range(B)])  # [B,4,128,2,S]
        wv8 = _pair8(Wv32.T * SW8)
        wl8 = _pair8(Wl32.T * SW8)
        wvot = np.ascontiguousarray((Wo32 @ Wv32).T.astype(mdt))
        wlot = np.ascontiguousarray((Wo32 @ Wl32).T.astype(mdt))
        return {"xt": xtc, "xt8": xt8, "wv8": wv8, "wl8": wl8,
                "wvot": wvot, "wlot": wlot, "bob": bob}
    wvt = np.ascontiguousarray(Wv32.T.astype(mdt))
    wlt = np.ascontiguousarray(Wl32.T.astype(mdt))
    wot = np.ascontiguousarray(Wo32.T.astype(mdt))
    return {"xt": xtc, "wvt": wvt, "wlt": wlt, "wot": wot, "bob": bob}


def _in_maps(tensors):
    per_batch = {"xt", "xt8"}
    return [
        {k: (v[c * BPC:(c + 1) * BPC] if k in per_batch else v)
         for k, v in tensors.items()}
        for c in range(NCORES)
    ]


DEFAULT_MODE = "f8p"


def kernel(vision_tokens, language_tokens, Wv, Wl, Wo, bo):
    from concourse.bass_utils import run_bass_kernel_spmd

    tensors = _host_prep(
        vision_tokens, language_tokens, Wv, Wl, Wo, bo, DEFAULT_MODE)
    nc = _get_program(repeat=1, mm_mode=DEFAULT_MODE)
    res = run_bass_kernel_spmd(nc, _in_maps(tensors), list(range(NCORES)))
    out = np.concatenate([res.results[c]["out"] for c in range(NCORES)], axis=0)
    return np.ascontiguousarray(out.astype(np.float32))

